# revision 37
# baseline (speedup 1.0000x reference)
"""Trainium2 Bass kernel for nn_Attention_89902255440825.

Single-layer attention block: QKV proj + per-head RMS("mult" variant) +
RoPE + GQA causal attention with softmax(scores * sqrt(HD)) + O proj.

Sharding (8 NeuronCores, tensor-parallel over heads):
  core c: q heads {2c, 2c+1}, kv head c//2, wo rows 256c:256c+256 ->
  partial [S,H] outputs (fp16), summed on host.

Precision (validated numerically + on HW):
  - q/k path (projections + scores) in fp32r: 1 cyc/row on the PE (fp16
    speed) with 12-bit mantissa; end-to-end emulation rel err ~8e-3 vs
    the 2e-2 gate (fp16 single-pass measured 1.8e-2 -- too close).
  - RMS sqrt via exp(0.5*ln(y)) (same act table as softmax Exp; measured
    1e-5 rel on HW; enters softmax as a per-row scale -> benign).
  - v / attn / O-proj path fp16 (contributes <~4e-4).

Schedule: projections run in 256-token psum chunks and are
software-pipelined with the attention blocks of the previous 512-token
quarter, so the PE-heavy projection overlaps the DVE/ACT-heavy softmax.
fp32r matmul accumulation chains corrupt each other when two chains
share a psum bank (measured on HW), so each projection chain owns a
full bank: q0|q1 then k|v reuse a 2-bank rotation.  PSUM banks:
prj 2 + scores 2 + transposes 1 + PV-accum 1 + O-proj 2 = 8; the idle
prj banks are borrowed for score chunks in the final (proj-free) flush.
Blocks with >=2 score chunks recompute scores (pass1 rowmax frees
banks, pass2 feeds exp straight from PSUM).  The causal diagonal mask
is preloaded into PSUM by the PE (ident @ cmask matmul).  attn_out
normalization runs on ACT (Copy with scale=1/l) and is transposed for
the O-proj lhsT on the PE; O-proj output staging alternates ACT/DVE
and is written back as one fp16 DMA per token block, interleaved two
blocks behind the attention pipeline.
"""
import numpy as np
from contextlib import ExitStack

import concourse.bass as bass
import concourse.bass_isa as bass_isa
import concourse.tile as tile
from concourse import bacc, mybir, bass_utils
from concourse.masks import make_identity
import concourse.hw_specs as _hw_specs
import concourse.bass_interp as _bass_interp

# Steer every Exp/Ln/Square/Copy/Identity activation to the one table set
# that contains them all; the default chooser alternates between
# exp_and_others and natural_log, reloading tables on every RMS<->softmax
# switch (25 x 1283ns).  Ids are positional, so only membership is edited.
_GAT_ORIG = _hw_specs.get_activation_tables


def _gat_one_table(arch):
    t = _GAT_ORIG(arch)
    A = mybir.ActivationFunctionType
    ours = {A.Exp, A.Ln, A.Square, A.Copy, A.Identity}
    return {k: (v if k == "natural_log_exp_and_others" else (v - ours))
            for k, v in t.items()}


for _m in (_hw_specs, bacc, _bass_interp):
    _m.get_activation_tables = _gat_one_table

S = 2048
H = 2048
HD = 128
NH = 16
NKV = 4
NCORES = 8
HPC = NH // NCORES          # q heads per core = 2
FQ = HPC * HD               # q features per core = 256
EPS = 1e-6
NEG = -30000.0              # causal mask additive constant (f32r exact)
F32 = mybir.dt.float32
F32R = mybir.dt.float32r
F16 = mybir.dt.float16
AX = mybir.AxisListType.X
AXY = mybir.AxisListType.XY
OP = mybir.AluOpType
ACTF = mybir.ActivationFunctionType

_prog_cache = {}


def _build(is_causal: bool):
    nc = bacc.Bacc("TRN2", target_bir_lowering=False, debug=False,
                   num_devices=NCORES)

    def din(name, shape, dt):
        return nc.dram_tensor(name, shape, dt, kind="ExternalInput").ap()

    xr_d = din("xr", [H, S], F32R)
    wq_d = din("wq", [H, FQ], F32R)
    wk_d = din("wk", [H, HD], F32R)
    wv_d = din("wv", [H, HD], F32R)
    wo_d = din("wo", [FQ, H], F16)
    cosq_d = din("cosq", [HD, S], F32)
    sinq_d = din("sinq", [HD, S], F32)   # rows 0:64 sign-flipped on host
    cosk_d = din("cosk", [HD, S], F32)
    sink_d = din("sink", [HD, S], F32)
    bqt_d = din("bqt", [HD, HPC], F32)
    bkt_d = din("bkt", [HD, 1], F32)
    bvt_d = din("bvt", [HD, 1], F32)
    if not is_causal:
        mask_d = din("maskadd", [S, S], F32)
    out_d = nc.dram_tensor("out", [S, H], F16, kind="ExternalOutput").ap()

    NKB = H // 128            # 16 contraction k-blocks
    NMB = S // 128            # 16 token blocks
    NCH = S // 512            # 4 512-chunks

    with tile.TileContext(nc) as tc, ExitStack() as ctx:
        const = ctx.enter_context(tc.tile_pool(name="const", bufs=1))
        wpool = ctx.enter_context(tc.tile_pool(name="wpool", bufs=1))
        big = ctx.enter_context(tc.tile_pool(name="big", bufs=1))
        xpool = ctx.enter_context(tc.tile_pool(name="xpool", bufs=5))
        btmp = ctx.enter_context(tc.tile_pool(name="btmp", bufs=2))
        cpool = ctx.enter_context(tc.tile_pool(name="cpool", bufs=2))
        psum = ctx.enter_context(tc.tile_pool(name="psum", bufs=1,
                                              space="PSUM"))

        # ---- constants ----
        ident16 = const.tile([128, 128], F16)
        make_identity(nc, ident16[:])
        ident_r = const.tile([128, 128], F32R)
        nc.vector.tensor_copy(ident_r[:], ident16[:])
        eps_q = const.tile([128, 1], F32)
        nc.vector.memset(eps_q[:], EPS * HD)
        eps_k = const.tile([128, 1], F32)
        nc.vector.memset(eps_k[:], EPS)
        if is_causal:
            cmask = const.tile([128, 4, 512], F32R)
            for r in range(4):
                cm_f = btmp.tile([128, 512], F32, tag="sq", name=f"cmf_{r}")
                nc.vector.memset(cm_f[:], 0.0)
                nc.gpsimd.affine_select(
                    out=cm_f[:], in_=cm_f[:],
                    compare_op=OP.is_ge, fill=NEG,
                    base=128 * r, channel_multiplier=1, pattern=[[-1, 512]],
                )
                nc.vector.tensor_copy(cmask[:, r, :], cm_f[:])

        # ---- weights / small inputs to SBUF ----
        def wtiles(dram, f, dt, nm):
            t = wpool.tile([128, NKB, f], dt, name=nm, tag=nm)
            nc.sync.dma_start(t[:], dram.rearrange("(t p) f -> p t f", p=128))
            return t

        wq_sb = wpool.tile([128, NKB, FQ], F32R, name="wq_sb", tag="wq_sb")

        def load_wq_split(_wi):
            nc.sync.dma_start(
                wq_sb[:, 4 * _wi:4 * (_wi + 1), :],
                wq_d[512 * _wi:512 * (_wi + 1), :].rearrange(
                    "(t p) f -> p t f", p=128))
        wk_sb = wpool.tile([128, NKB, HD], F32R, name="wk_sb", tag="wk_sb")
        wv_sb = wpool.tile([128, NKB, HD], F32R, name="wv_sb", tag="wv_sb")
        wkv_loaded = [False]
        wo_sb = wpool.tile([128, HPC, H], F16)
        cosq_sb = wpool.tile([HD, S], F32)
        sinq_sb = wpool.tile([HD, S], F32)
        cosk_sb = wpool.tile([HD, S], F32)
        sink_sb = wpool.tile([HD, S], F32)
        bqt_sb = wpool.tile([HD, HPC], F32)
        nc.sync.dma_start(bqt_sb[:], bqt_d)
        bkt_sb = wpool.tile([HD, 1], F32)
        nc.sync.dma_start(bkt_sb[:], bkt_d)
        bvt_sb = wpool.tile([HD, 1], F32)
        nc.sync.dma_start(bvt_sb[:], bvt_d)

        # ---- persistent activations ----
        vt16 = big.tile([128, S], F16)            # v feature-major fp16
        v_sb = big.tile([128, NMB, 128], F16)     # v token-major fp16
        qf = big.tile([128, HPC, S], F32R)        # roped+scaled q
        kf = big.tile([128, S], F32R)

        qst = {}
        kst = {}
        prj = {}

        # ============ projection chunk: 256 tokens, q0/q1/k/v ============
        # fp32r matmul accumulation chains must own a full psum bank: two
        # [128,256] chains run in parallel banks (q0|q1, then k|v reusing
        # the banks after the bias drain).
        def emit_proj_chunk(c):
            mq = c // 2
            if c % 2 == 0:
                qst[mq] = btmp.tile([128, HPC, 512], F32, tag="qst",
                                    name=f"qst_{mq}")
                kst[mq] = btmp.tile([128, 512], F32, tag="kst",
                                    name=f"kst_{mq}")
            cs = slice(c * 256, (c + 1) * 256)
            hs = slice((c % 2) * 256, (c % 2) * 256 + 256)
            x_ts = []
            if c == 0:
                for _wi in range(4):
                    load_wq_split(_wi)
            for kb4 in range(4):
                x_t = xpool.tile([128, 4, 256], F32R, tag="xt",
                                 name=f"x_{c}_{kb4}")
                nc.sync.dma_start(
                    x_t[:],
                    xr_d[kb4 * 512:(kb4 + 1) * 512, cs].rearrange(
                        "(t p) m -> p t m", p=128))
                x_ts.append(x_t)

            def chain(w_ap_fn, nm):
                pc = psum.tile([128, 256], F32, tag="prj", bufs=2,
                               name=f"prj_{c}_{nm}")
                for kb in range(NKB):
                    nc.tensor.matmul(pc[:], w_ap_fn(kb),
                                     x_ts[kb // 4][:, kb % 4, :],
                                     start=(kb == 0), stop=(kb == NKB - 1))
                return pc

            p0 = chain(lambda kb: wq_sb[:, kb, 0:128], "q0")
            if not wkv_loaded[0]:
                wkv_loaded[0] = True
                nc.sync.dma_start(
                    wk_sb[:], wk_d.rearrange("(t p) f -> p t f", p=128))
                nc.sync.dma_start(
                    wv_sb[:], wv_d.rearrange("(t p) f -> p t f", p=128))
            p1 = chain(lambda kb: wq_sb[:, kb, 128:256], "q1")
            nc.scalar.add(qst[mq][:, 0, hs], p0[:], bqt_sb[:, 0:1])
            nc.scalar.add(qst[mq][:, 1, hs], p1[:], bqt_sb[:, 1:2])
            p2 = chain(lambda kb: wk_sb[:, kb, :], "k")
            p3 = chain(lambda kb: wv_sb[:, kb, :], "v")
            nc.scalar.add(kst[mq][:, hs], p2[:], bkt_sb[:])
            nc.scalar.add(vt16[:, cs], p3[:], bvt_sb[:])

        # ============ RMS("mult") + RoPE for one 512-token quarter =======
        def emit_rope(mq):
            ms = slice(mq * 512, (mq + 1) * 512)
            specs = [
                (kst[mq][:], kf[:, ms], 1.0 / HD, eps_k, cosk_sb, sink_sb),
                (qst[mq][:, 0], qf[:, 0, ms], 1.0, eps_q,
                 cosq_sb, sinq_sb),
                (qst[mq][:, 1], qf[:, 1, ms], 1.0, eps_q,
                 cosq_sb, sinq_sb),
            ]
            for sn, (bsrc, dst, sc, ep, cos_sb, sin_sb) in enumerate(specs):
                sq = btmp.tile([128, 512], F32, tag="sq",
                               name=f"sq_{mq}_{sn}")
                nc.scalar.activation(sq[:], bsrc, ACTF.Square)
                pr = btmp.tile([128, 512], F32, tag="pr",
                               name=f"pr_{mq}_{sn}")
                nc.gpsimd.partition_all_reduce(pr[:], sq[:], channels=128,
                                               reduce_op=bass_isa.ReduceOp.add)
                # s = sqrt(pr*sc + ep) = exp(0.5*ln(pr*sc + ep)); the ln/exp
                # table also serves softmax Exp so no act-table thrash
                ln_t = btmp.tile([128, 512], F32, tag="lnx",
                                 name=f"ln_{mq}_{sn}")
                nc.scalar.activation(ln_t[:], pr[:], ACTF.Ln,
                                     bias=ep[:], scale=sc)
                s_bc = btmp.tile([128, 512], F32, tag="lnx",
                                 name=f"sbc_{mq}_{sn}")
                nc.scalar.activation(s_bc[:], ln_t[:], ACTF.Exp, scale=0.5)
                rot = btmp.tile([128, 512], F32, tag="rot",
                                name=f"rot_{mq}_{sn}")
                nc.vector.tensor_copy(rot[0:64, :], bsrc[64:128, :])
                nc.vector.tensor_copy(rot[64:128, :], bsrc[0:64, :])
                qr = btmp.tile([128, 512], F32, tag="qr",
                               name=f"qr_{mq}_{sn}")
                nc.gpsimd.tensor_mul(qr[:], bsrc, cos_sb[:, ms])
                nc.gpsimd.tensor_mul(rot[:], rot[:], sin_sb[:, ms])
                nc.gpsimd.tensor_add(qr[:], qr[:], rot[:])
                nc.vector.tensor_mul(dst, qr[:], s_bc[:])

        def emit_vtrans(mq):
            for mb in range(mq * 4, mq * 4 + 4):
                pvt = psum.tile([128, 128], F16, tag="acc2", bufs=2,
                                name=f"pvt_{mb}")
                nc.tensor.transpose(pvt[:], vt16[:, mb * 128:(mb + 1) * 128],
                                    ident16[:])
                nc.vector.tensor_copy(v_sb[:, mb], pvt[:])

        # ================= attention + O proj ===========================
        prev = [None]
        oproj_pending = []

        def oproj_step(state):
            """Emit the next O-proj chunk of the pending block, if any."""
            if not state:
                return
            j, attn16, ob, nh_ = state[0]
            if nh_ >= 4:
                nc.sync.dma_start(out_d[j * 128:(j + 1) * 128, :], ob[:])
                state.pop(0)
                return
            ns = slice(nh_ * 512, (nh_ + 1) * 512)
            po = psum.tile([128, 512], F32, tag="acc2", bufs=2,
                           name=f"po_{j}_{nh_}")
            nc.tensor.matmul(po[:], attn16[:, 0], wo_sb[:, 0, ns],
                             start=True, stop=False)
            nc.tensor.matmul(po[:], attn16[:, 1], wo_sb[:, 1, ns],
                             start=False, stop=True)
            if nh_ % 2 == 0:
                nc.scalar.copy(ob[:, ns], po[:])
            else:
                nc.vector.tensor_copy(ob[:, ns], po[:])
            state[0] = (j, attn16, ob, nh_ + 1)

        def emit_oproj_all(state):
            while state:
                oproj_step(state)

        def emit_tail(st):
            i, nchunks, p16s, lpartss = st
            ostate = []
            if len(oproj_pending) >= 2:
                j, attn16 = oproj_pending.pop(0)
                ob = cpool.tile([128, H], F16, tag="ob", name=f"ob_{j}")
                ostate.append((j, attn16, ob, 0))
                oproj_step(ostate)
                oproj_step(ostate)
            out_ps2 = psum.tile([128, HPC, 128], F32, tag="opv", bufs=1,
                                name=f"opv_{i}")
            last_nkb = i if is_causal else NMB - 1
            ng = 0
            for h in range(HPC):
                nchk = len(p16s[h])
                for pj in range((nchk + 1) // 2):
                    ng += 1
                    if ng % 2 == 0:
                        oproj_step(ostate)
                    ncjs = [c for c in (2 * pj, 2 * pj + 1) if c < nchk]
                    nbs = [min(4, last_nkb + 1 - c * 4) for c in ncjs]
                    nbt = sum(nbs)
                    ps_t8 = psum.tile([128, 8, 128], F16, tag="pt4",
                                      bufs=1, name=f"pt8_{i}_{h}_{pj}")
                    bo_ = 0
                    for c, nb in zip(ncjs, nbs):
                        for b in range(nb):
                            nc.tensor.transpose(
                                ps_t8[:, bo_ + b, :],
                                p16s[h][c][:, b * 128:(b + 1) * 128],
                                ident16[:])
                        bo_ += nb
                    pt_sb = cpool.tile([128, 8, 128], F16, tag="pt_sb",
                                       bufs=4, name=f"pt_{i}_{h}_{pj}")
                    nc.vector.tensor_copy(pt_sb[:, 0:nbt, :],
                                          ps_t8[:, 0:nbt, :])
                    bo_ = 0
                    for c, nb in zip(ncjs, nbs):
                        for b in range(nb):
                            nkb = c * 4 + b
                            nc.tensor.matmul(out_ps2[:, h, :],
                                             pt_sb[:, bo_ + b, :],
                                             v_sb[:, nkb],
                                             start=(nkb == 0),
                                             stop=(nkb == last_nkb))
                        bo_ += nb
            emit_oproj_all(ostate)
            at_t = cpool.tile([128, HPC, 128], F16, tag="at_t",
                              name=f"at_{i}")
            attn16 = cpool.tile([128, HPC, 128], F16, tag="attn16",
                                name=f"attn16_{i}")
            for h in range(HPC):
                lsum = cpool.tile([128, 1], F32, tag="lsum",
                                  name=f"lsum_{i}_{h}")
                nc.vector.reduce_sum(lsum[:], lpartss[h][:, 0:nchunks],
                                     axis=AX)
                linv = cpool.tile([128, 1], F32, tag="linv",
                                  name=f"linv_{i}_{h}")
                nc.vector.reciprocal(linv[:], lsum[:])
                nc.scalar.activation(at_t[:, h, :], out_ps2[:, h, :],
                                     ACTF.Copy, scale=linv[:])
                pat = psum.tile([128, 128], F16, tag="pt4", bufs=1,
                                name=f"pat_{i}_{h}")
                nc.tensor.transpose(pat[:], at_t[:, h, :], ident16[:])
                nc.vector.tensor_copy(attn16[:, h, :], pat[:])
            oproj_pending.append((i, attn16))

        def emit_attn_block(i):
            nchunks = (i // 4 + 1) if is_causal else NCH
            recompute = nchunks >= 2
            p16s = [[], []]
            lpartss = []

            def score_chunk(h, ncj, tag_sfx):
                ks = slice(ncj * 512, (ncj + 1) * 512)
                flush = i >= 12
                tg = "prj" if (flush and ncj % 2 == 1) else "sch"
                ps_c = psum.tile([128, 512], F32, tag=tg, bufs=2,
                                 name=f"sc_{i}_{h}_{ncj}{tag_sfx}")
                diag = is_causal and ncj == i // 4
                if diag:
                    nc.tensor.matmul(ps_c[:], ident_r[:], cmask[:, i % 4, :],
                                     start=True, stop=False)
                nc.tensor.matmul(ps_c[:], qf[:, h, i * 128:(i + 1) * 128],
                                 kf[:, ks], start=not diag, stop=True)
                if not is_causal:
                    mload = cpool.tile([128, 512], F32, tag="mload",
                                       bufs=3, name=f"ml_{i}_{h}_{ncj}{tag_sfx}")
                    nc.sync.dma_start(
                        mload[:], mask_d[i * 128:(i + 1) * 128, ks])
                    nc.vector.tensor_add(ps_c[:], ps_c[:], mload[:])
                return ps_c

            # pass 1, heads interleaved: scores -> per-chunk rowmax; psum
            # chunk banks free right after the reduce (recompute mode)
            pmxs, negms, chunks = [], [], {}
            for h in range(HPC):
                pmxs.append(cpool.tile([128, 4], F32, tag="pmx",
                                       name=f"pmx_{i}_{h}")
                            if nchunks > 1 else None)
            for ncj in range(nchunks):
                for h in range(HPC):
                    ps_c = score_chunk(h, ncj, "a")
                    if nchunks > 1:
                        nc.vector.reduce_max(pmxs[h][:, ncj:ncj + 1], ps_c[:],
                                             axis=AX)
                    if not recompute:
                        chunks[(h, ncj)] = ps_c
            for h in range(HPC):
                negm = cpool.tile([128, 1], F32, tag="negm",
                                  name=f"negm_{i}_{h}")
                if nchunks > 1:
                    nc.vector.reduce_max(negm[:], pmxs[h][:, 0:nchunks],
                                         axis=AX, negate=True)
                else:
                    nc.vector.reduce_max(negm[:], chunks[(h, 0)][:],
                                         axis=AX, negate=True)
                negms.append(negm)
            if prev[0] is not None:
                emit_tail(prev[0])
            # pass 2, heads interleaved: (recomputed) scores -> exp
            lpartss = [cpool.tile([128, 4], F32, tag="lparts",
                                  name=f"lp_{i}_{h}") for h in range(HPC)]
            for ncj in range(nchunks):
                for h in range(HPC):
                    ps_c = score_chunk(h, ncj, "b") if recompute \
                        else chunks[(h, ncj)]
                    p16 = cpool.tile([128, 512], F16, tag="p16", bufs=10,
                                     name=f"p16_{i}_{h}_{ncj}")
                    nc.scalar.activation(p16[:], ps_c[:],
                                         ACTF.Exp, bias=negms[h][:],
                                         scale=1.0,
                                         accum_out=lpartss[h][:, ncj:ncj + 1])
                    p16s[h].append(p16)
            prev[0] = (i, nchunks, p16s, lpartss)

        # ================= main schedule ================================
        for mq in range(4):
            emit_proj_chunk(2 * mq)
            if mq == 1:
                nc.sync.dma_start(
                    wo_sb[:], wo_d.rearrange("(t p) f -> p t f", p=128))
            if mq > 0:
                emit_attn_block(4 * (mq - 1))
                emit_attn_block(4 * (mq - 1) + 1)
            emit_proj_chunk(2 * mq + 1)
            if mq == 0:
                nc.sync.dma_start(cosk_sb[:], cosk_d)
                nc.sync.dma_start(sink_sb[:], sink_d)
                nc.sync.dma_start(cosq_sb[:], cosq_d)
                nc.sync.dma_start(sinq_sb[:], sinq_d)
            if mq > 0:
                emit_attn_block(4 * (mq - 1) + 2)
                emit_attn_block(4 * (mq - 1) + 3)
            emit_rope(mq)
            emit_vtrans(mq)
        for i in range(12, 16):
            emit_attn_block(i)
        emit_tail(prev[0])
        while oproj_pending:
            j, attn16 = oproj_pending.pop(0)
            ob = cpool.tile([128, H], F16, tag="ob", name=f"ob_{j}")
            emit_oproj_all([(j, attn16, ob, 0)])

    nc.compile()
    return nc


def _f32r(a):
    u = np.ascontiguousarray(a, np.float32).view(np.uint32)
    u = (u + np.uint32(0x800)) & np.uint32(0xFFFFF000)
    return u.view(np.float32)


def kernel(**inputs):
    x = np.asarray(inputs["x"], np.float32)
    cos = np.asarray(inputs["cos"], np.float32)
    sin = np.asarray(inputs["sin"], np.float32)
    am = np.asarray(inputs["attention_mask"]).reshape(S, S).astype(bool)
    wq = np.asarray(inputs["wq"], np.float32)
    bq = np.asarray(inputs["bq"], np.float32)
    wk = np.asarray(inputs["wk"], np.float32)
    bk = np.asarray(inputs["bk"], np.float32)
    wv = np.asarray(inputs["wv"], np.float32)
    bv = np.asarray(inputs["bv"], np.float32)
    wo = np.asarray(inputs["wo"], np.float32)
    bo = np.asarray(inputs["bo"], np.float32)
    qn = np.asarray(inputs["q_norm_w"], np.float32)
    kn = np.asarray(inputs["k_norm_w"], np.float32)

    assert x.shape == (1, S, H)
    is_causal = bool(
        (am == np.triu(np.ones((S, S), dtype=bool), k=1)).all())

    key = is_causal
    if key not in _prog_cache:
        _prog_cache[key] = _build(is_causal)
    nc = _prog_cache[key]

    xr = _f32r(x[0].T)
    cosT = cos.T
    sinT = sin.T
    rolled_q = np.roll(qn, -64)     # rot(q*qn)[i] = rot(q)[i] * qn[(i+64)%128]
    rolled_k = np.roll(kn, -64)
    # rope via rot'=[x2;x1] (plain swap): fold the sign of the first half
    # of sin into the table
    sgn = np.ones((HD, 1), np.float32)
    sgn[0:HD // 2] = -1.0
    cosq = np.ascontiguousarray(cosT * qn[:, None])
    sinq = np.ascontiguousarray(sinT * rolled_q[:, None] * sgn)
    cosk = np.ascontiguousarray(cosT * kn[:, None])
    sink = np.ascontiguousarray(sinT * rolled_k[:, None] * sgn)
    if not is_causal:
        maskadd = np.where(am, np.float32(NEG), np.float32(0.0))

    in_maps = []
    for c in range(NCORES):
        fq = slice(c * FQ, (c + 1) * FQ)
        g = c // 2
        fk = slice(g * HD, (g + 1) * HD)
        m = dict(
            xr=xr,
            wq=_f32r(wq[:, fq]),
            wk=_f32r(wk[:, fk]),
            wv=_f32r(wv[:, fk]),
            wo=np.ascontiguousarray(wo[fq, :].astype(np.float16)),
            cosq=cosq, sinq=sinq, cosk=cosk, sink=sink,
            bqt=np.ascontiguousarray(bq[fq].reshape(HPC, HD).T),
            bkt=np.ascontiguousarray(bk[fk].reshape(1, HD).T),
            bvt=np.ascontiguousarray(bv[fk].reshape(1, HD).T),
        )
        if not is_causal:
            m["maskadd"] = maskadd
        in_maps.append(m)

    res = bass_utils.run_bass_kernel_spmd(nc, in_maps,
                                          core_ids=list(range(NCORES)))
    acc = np.zeros((S, H), np.float64)
    for c in range(NCORES):
        acc += res.results[c]["out"]
    out = (acc + bo[None, :]).astype(np.float32)
    return out.reshape(1, S, H)


# revision 38
# speedup vs baseline: 1.0079x; 1.0079x over previous
"""Trainium2 Bass kernel for nn_Attention_89902255440825.

Single-layer attention block: QKV proj + per-head RMS("mult" variant) +
RoPE + GQA causal attention with softmax(scores * sqrt(HD)) + O proj.

Sharding (8 NeuronCores, tensor-parallel over heads):
  core c: q heads {2c, 2c+1}, kv head c//2, wo rows 256c:256c+256 ->
  partial [S,H] outputs (fp16), summed on host.

Precision (validated numerically + on HW):
  - q/k path (projections + scores) in fp32r: 1 cyc/row on the PE (fp16
    speed) with 12-bit mantissa; end-to-end emulation rel err ~8e-3 vs
    the 2e-2 gate (fp16 single-pass measured 1.8e-2 -- too close).
  - RMS sqrt via exp(0.5*ln(y)) (same act table as softmax Exp; measured
    1e-5 rel on HW; enters softmax as a per-row scale -> benign).
  - v / attn / O-proj path fp16 (contributes <~4e-4).

Schedule: projections run in 256-token psum chunks and are
software-pipelined with the attention blocks of the previous 512-token
quarter, so the PE-heavy projection overlaps the DVE/ACT-heavy softmax.
fp32r matmul accumulation chains corrupt each other when two chains
share a psum bank (measured on HW), so each projection chain owns a
full bank: q0|q1 then k|v reuse a 2-bank rotation.  PSUM banks:
prj 2 + scores 2 + transposes 1 + PV-accum 1 + O-proj 2 = 8; the idle
prj banks are borrowed for score chunks in the final (proj-free) flush.
Blocks with >=2 score chunks recompute scores (pass1 rowmax frees
banks, pass2 feeds exp straight from PSUM).  The causal diagonal mask
is preloaded into PSUM by the PE (ident @ cmask matmul).  attn_out
normalization runs on ACT (Copy with scale=1/l) and is transposed for
the O-proj lhsT on the PE; O-proj output staging alternates ACT/DVE
and is written back as one fp16 DMA per token block, interleaved two
blocks behind the attention pipeline.
"""
import numpy as np
from contextlib import ExitStack

import concourse.bass as bass
import concourse.bass_isa as bass_isa
import concourse.tile as tile
from concourse import bacc, mybir, bass_utils
from concourse.masks import make_identity
import concourse.hw_specs as _hw_specs
import concourse.bass_interp as _bass_interp

# Steer every Exp/Ln/Square/Copy/Identity activation to the one table set
# that contains them all; the default chooser alternates between
# exp_and_others and natural_log, reloading tables on every RMS<->softmax
# switch (25 x 1283ns).  Ids are positional, so only membership is edited.
_GAT_ORIG = _hw_specs.get_activation_tables


def _gat_one_table(arch):
    t = _GAT_ORIG(arch)
    A = mybir.ActivationFunctionType
    ours = {A.Exp, A.Ln, A.Square, A.Copy, A.Identity}
    return {k: (v if k == "natural_log_exp_and_others" else (v - ours))
            for k, v in t.items()}


for _m in (_hw_specs, bacc, _bass_interp):
    _m.get_activation_tables = _gat_one_table

S = 2048
H = 2048
HD = 128
NH = 16
NKV = 4
NCORES = 8
HPC = NH // NCORES          # q heads per core = 2
FQ = HPC * HD               # q features per core = 256
EPS = 1e-6
NEG = -30000.0              # causal mask additive constant (f32r exact)
F32 = mybir.dt.float32
F32R = mybir.dt.float32r
F16 = mybir.dt.float16
AX = mybir.AxisListType.X
AXY = mybir.AxisListType.XY
OP = mybir.AluOpType
ACTF = mybir.ActivationFunctionType

_prog_cache = {}


def _build(is_causal: bool):
    nc = bacc.Bacc("TRN2", target_bir_lowering=False, debug=False,
                   num_devices=NCORES)

    def din(name, shape, dt):
        return nc.dram_tensor(name, shape, dt, kind="ExternalInput").ap()

    xr_d = din("xr", [H, S], F32R)
    wq_d = din("wq", [H, FQ], F32R)
    wk_d = din("wk", [H, HD], F32R)
    wv_d = din("wv", [H, HD], F32R)
    wo_d = din("wo", [FQ, H], F16)
    cosq_d = din("cosq", [HD, S], F32)
    sinq_d = din("sinq", [HD, S], F32)   # rows 0:64 sign-flipped on host
    cosk_d = din("cosk", [HD, S], F32)
    sink_d = din("sink", [HD, S], F32)
    bqt_d = din("bqt", [HD, HPC], F32)
    bkt_d = din("bkt", [HD, 1], F32)
    bvt_d = din("bvt", [HD, 1], F32)
    if not is_causal:
        mask_d = din("maskadd", [S, S], F32)
    out_d = nc.dram_tensor("out", [S, H], F16, kind="ExternalOutput").ap()

    NKB = H // 128            # 16 contraction k-blocks
    NMB = S // 128            # 16 token blocks
    NCH = S // 512            # 4 512-chunks

    with tile.TileContext(nc) as tc, ExitStack() as ctx:
        const = ctx.enter_context(tc.tile_pool(name="const", bufs=1))
        wpool = ctx.enter_context(tc.tile_pool(name="wpool", bufs=1))
        big = ctx.enter_context(tc.tile_pool(name="big", bufs=1))
        xpool = ctx.enter_context(tc.tile_pool(name="xpool", bufs=5))
        btmp = ctx.enter_context(tc.tile_pool(name="btmp", bufs=2))
        cpool = ctx.enter_context(tc.tile_pool(name="cpool", bufs=2))
        psum = ctx.enter_context(tc.tile_pool(name="psum", bufs=1,
                                              space="PSUM"))

        # ---- constants ----
        ident16 = const.tile([128, 128], F16)
        make_identity(nc, ident16[:])
        ident_r = const.tile([128, 128], F32R)
        nc.vector.tensor_copy(ident_r[:], ident16[:])
        eps_q = const.tile([128, 1], F32)
        nc.vector.memset(eps_q[:], EPS * HD)
        eps_k = const.tile([128, 1], F32)
        nc.vector.memset(eps_k[:], EPS)
        if is_causal:
            cmask = const.tile([128, 4, 512], F32R)
            for r in range(4):
                cm_f = btmp.tile([128, 512], F32, tag="sq", name=f"cmf_{r}")
                nc.vector.memset(cm_f[:], 0.0)
                nc.gpsimd.affine_select(
                    out=cm_f[:], in_=cm_f[:],
                    compare_op=OP.is_ge, fill=NEG,
                    base=128 * r, channel_multiplier=1, pattern=[[-1, 512]],
                )
                nc.vector.tensor_copy(cmask[:, r, :], cm_f[:])

        # ---- weights / small inputs to SBUF ----
        def wtiles(dram, f, dt, nm):
            t = wpool.tile([128, NKB, f], dt, name=nm, tag=nm)
            nc.sync.dma_start(t[:], dram.rearrange("(t p) f -> p t f", p=128))
            return t

        wq_sb = wpool.tile([128, NKB, FQ], F32R, name="wq_sb", tag="wq_sb")
        for _wi in range(4):
            nc.sync.dma_start(
                wq_sb[:, 4 * _wi:4 * (_wi + 1), :],
                wq_d[512 * _wi:512 * (_wi + 1), :].rearrange(
                    "(t p) f -> p t f", p=128))
        wk_sb = wpool.tile([128, NKB, HD], F32R, name="wk_sb", tag="wk_sb")
        wv_sb = wpool.tile([128, NKB, HD], F32R, name="wv_sb", tag="wv_sb")
        wkv_loaded = [False]
        wo_sb = wpool.tile([128, HPC, H], F16)
        cosq_sb = wpool.tile([HD, S], F32)
        sinq_sb = wpool.tile([HD, S], F32)
        cosk_sb = wpool.tile([HD, S], F32)
        sink_sb = wpool.tile([HD, S], F32)
        bqt_sb = wpool.tile([HD, HPC], F32)
        nc.sync.dma_start(bqt_sb[:], bqt_d)
        bkt_sb = wpool.tile([HD, 1], F32)
        nc.sync.dma_start(bkt_sb[:], bkt_d)
        bvt_sb = wpool.tile([HD, 1], F32)
        nc.sync.dma_start(bvt_sb[:], bvt_d)

        # ---- persistent activations ----
        vt16 = big.tile([128, S], F16)            # v feature-major fp16
        v_sb = big.tile([128, NMB, 128], F16)     # v token-major fp16
        qf = big.tile([128, HPC, S], F32R)        # roped+scaled q
        kf = big.tile([128, S], F32R)

        qst = {}
        kst = {}
        prj = {}

        # ============ projection chunk: 256 tokens, q0/q1/k/v ============
        # fp32r matmul accumulation chains must own a full psum bank: two
        # [128,256] chains run in parallel banks (q0|q1, then k|v reusing
        # the banks after the bias drain).
        def emit_proj_chunk(c):
            mq = c // 2
            if c % 2 == 0:
                qst[mq] = btmp.tile([128, HPC, 512], F32, tag="qst",
                                    name=f"qst_{mq}")
                kst[mq] = btmp.tile([128, 512], F32, tag="kst",
                                    name=f"kst_{mq}")
            cs = slice(c * 256, (c + 1) * 256)
            hs = slice((c % 2) * 256, (c % 2) * 256 + 256)
            x_ts = []
            for kb4 in range(4):
                x_t = xpool.tile([128, 4, 256], F32R, tag="xt",
                                 name=f"x_{c}_{kb4}")
                nc.sync.dma_start(
                    x_t[:],
                    xr_d[kb4 * 512:(kb4 + 1) * 512, cs].rearrange(
                        "(t p) m -> p t m", p=128))
                x_ts.append(x_t)

            def chain(w_ap_fn, nm):
                pc = psum.tile([128, 256], F32, tag="prj", bufs=2,
                               name=f"prj_{c}_{nm}")
                for kb in range(NKB):
                    nc.tensor.matmul(pc[:], w_ap_fn(kb),
                                     x_ts[kb // 4][:, kb % 4, :],
                                     start=(kb == 0), stop=(kb == NKB - 1))
                return pc

            p0 = chain(lambda kb: wq_sb[:, kb, 0:128], "q0")
            if not wkv_loaded[0]:
                wkv_loaded[0] = True
                nc.sync.dma_start(
                    wk_sb[:], wk_d.rearrange("(t p) f -> p t f", p=128))
                nc.sync.dma_start(
                    wv_sb[:], wv_d.rearrange("(t p) f -> p t f", p=128))
            p1 = chain(lambda kb: wq_sb[:, kb, 128:256], "q1")
            nc.scalar.add(qst[mq][:, 0, hs], p0[:], bqt_sb[:, 0:1])
            nc.scalar.add(qst[mq][:, 1, hs], p1[:], bqt_sb[:, 1:2])
            p2 = chain(lambda kb: wk_sb[:, kb, :], "k")
            p3 = chain(lambda kb: wv_sb[:, kb, :], "v")
            nc.scalar.add(kst[mq][:, hs], p2[:], bkt_sb[:])
            nc.scalar.add(vt16[:, cs], p3[:], bvt_sb[:])

        # ============ RMS("mult") + RoPE for one 512-token quarter =======
        def emit_rope(mq):
            ms = slice(mq * 512, (mq + 1) * 512)
            specs = [
                (kst[mq][:], kf[:, ms], 1.0 / HD, eps_k, cosk_sb, sink_sb),
                (qst[mq][:, 0], qf[:, 0, ms], 1.0, eps_q,
                 cosq_sb, sinq_sb),
                (qst[mq][:, 1], qf[:, 1, ms], 1.0, eps_q,
                 cosq_sb, sinq_sb),
            ]
            for sn, (bsrc, dst, sc, ep, cos_sb, sin_sb) in enumerate(specs):
                sq = btmp.tile([128, 512], F32, tag="sq",
                               name=f"sq_{mq}_{sn}")
                nc.scalar.activation(sq[:], bsrc, ACTF.Square)
                pr = btmp.tile([128, 512], F32, tag="pr",
                               name=f"pr_{mq}_{sn}")
                nc.gpsimd.partition_all_reduce(pr[:], sq[:], channels=128,
                                               reduce_op=bass_isa.ReduceOp.add)
                # s = sqrt(pr*sc + ep) = exp(0.5*ln(pr*sc + ep)); the ln/exp
                # table also serves softmax Exp so no act-table thrash
                ln_t = btmp.tile([128, 512], F32, tag="lnx",
                                 name=f"ln_{mq}_{sn}")
                nc.scalar.activation(ln_t[:], pr[:], ACTF.Ln,
                                     bias=ep[:], scale=sc)
                s_bc = btmp.tile([128, 512], F32, tag="lnx",
                                 name=f"sbc_{mq}_{sn}")
                nc.scalar.activation(s_bc[:], ln_t[:], ACTF.Exp, scale=0.5)
                rot = btmp.tile([128, 512], F32, tag="rot",
                                name=f"rot_{mq}_{sn}")
                nc.vector.tensor_copy(rot[0:64, :], bsrc[64:128, :])
                nc.vector.tensor_copy(rot[64:128, :], bsrc[0:64, :])
                qr = btmp.tile([128, 512], F32, tag="qr",
                               name=f"qr_{mq}_{sn}")
                nc.gpsimd.tensor_mul(qr[:], bsrc, cos_sb[:, ms])
                nc.gpsimd.tensor_mul(rot[:], rot[:], sin_sb[:, ms])
                nc.gpsimd.tensor_add(qr[:], qr[:], rot[:])
                nc.vector.tensor_mul(dst, qr[:], s_bc[:])

        def emit_vtrans(mq):
            for mb in range(mq * 4, mq * 4 + 4):
                pvt = psum.tile([128, 128], F16, tag="acc2", bufs=2,
                                name=f"pvt_{mb}")
                nc.tensor.transpose(pvt[:], vt16[:, mb * 128:(mb + 1) * 128],
                                    ident16[:])
                nc.vector.tensor_copy(v_sb[:, mb], pvt[:])

        # ================= attention + O proj ===========================
        prev = [None]
        oproj_pending = []

        def oproj_step(state):
            """Emit the next O-proj chunk of the pending block, if any."""
            if not state:
                return
            j, attn16, ob, nh_ = state[0]
            if nh_ >= 4:
                nc.sync.dma_start(out_d[j * 128:(j + 1) * 128, :], ob[:])
                state.pop(0)
                return
            ns = slice(nh_ * 512, (nh_ + 1) * 512)
            po = psum.tile([128, 512], F32, tag="acc2", bufs=2,
                           name=f"po_{j}_{nh_}")
            nc.tensor.matmul(po[:], attn16[:, 0], wo_sb[:, 0, ns],
                             start=True, stop=False)
            nc.tensor.matmul(po[:], attn16[:, 1], wo_sb[:, 1, ns],
                             start=False, stop=True)
            if nh_ % 2 == 0:
                nc.scalar.copy(ob[:, ns], po[:])
            else:
                nc.vector.tensor_copy(ob[:, ns], po[:])
            state[0] = (j, attn16, ob, nh_ + 1)

        def emit_oproj_all(state):
            while state:
                oproj_step(state)

        def emit_tail(st):
            i, nchunks, p16s, lpartss = st
            ostate = []
            if len(oproj_pending) >= 2:
                j, attn16 = oproj_pending.pop(0)
                ob = cpool.tile([128, H], F16, tag="ob", name=f"ob_{j}")
                ostate.append((j, attn16, ob, 0))
                oproj_step(ostate)
                oproj_step(ostate)
            out_ps2 = psum.tile([128, HPC, 128], F32, tag="opv", bufs=1,
                                name=f"opv_{i}")
            last_nkb = i if is_causal else NMB - 1
            ng = 0
            for h in range(HPC):
                nchk = len(p16s[h])
                for pj in range((nchk + 1) // 2):
                    ng += 1
                    if ng % 2 == 0:
                        oproj_step(ostate)
                    ncjs = [c for c in (2 * pj, 2 * pj + 1) if c < nchk]
                    nbs = [min(4, last_nkb + 1 - c * 4) for c in ncjs]
                    nbt = sum(nbs)
                    ps_t8 = psum.tile([128, 8, 128], F16, tag="pt4",
                                      bufs=1, name=f"pt8_{i}_{h}_{pj}")
                    bo_ = 0
                    for c, nb in zip(ncjs, nbs):
                        for b in range(nb):
                            nc.tensor.transpose(
                                ps_t8[:, bo_ + b, :],
                                p16s[h][c][:, b * 128:(b + 1) * 128],
                                ident16[:])
                        bo_ += nb
                    pt_sb = cpool.tile([128, 8, 128], F16, tag="pt_sb",
                                       bufs=4, name=f"pt_{i}_{h}_{pj}")
                    nc.vector.tensor_copy(pt_sb[:, 0:nbt, :],
                                          ps_t8[:, 0:nbt, :])
                    bo_ = 0
                    for c, nb in zip(ncjs, nbs):
                        for b in range(nb):
                            nkb = c * 4 + b
                            nc.tensor.matmul(out_ps2[:, h, :],
                                             pt_sb[:, bo_ + b, :],
                                             v_sb[:, nkb],
                                             start=(nkb == 0),
                                             stop=(nkb == last_nkb))
                        bo_ += nb
            emit_oproj_all(ostate)
            at_t = cpool.tile([128, HPC, 128], F16, tag="at_t",
                              name=f"at_{i}")
            attn16 = cpool.tile([128, HPC, 128], F16, tag="attn16",
                                name=f"attn16_{i}")
            for h in range(HPC):
                lsum = cpool.tile([128, 1], F32, tag="lsum",
                                  name=f"lsum_{i}_{h}")
                nc.vector.reduce_sum(lsum[:], lpartss[h][:, 0:nchunks],
                                     axis=AX)
                linv = cpool.tile([128, 1], F32, tag="linv",
                                  name=f"linv_{i}_{h}")
                nc.vector.reciprocal(linv[:], lsum[:])
                nc.scalar.activation(at_t[:, h, :], out_ps2[:, h, :],
                                     ACTF.Copy, scale=linv[:])
                pat = psum.tile([128, 128], F16, tag="pt4", bufs=1,
                                name=f"pat_{i}_{h}")
                nc.tensor.transpose(pat[:], at_t[:, h, :], ident16[:])
                nc.vector.tensor_copy(attn16[:, h, :], pat[:])
            oproj_pending.append((i, attn16))

        def emit_attn_block(i):
            nchunks = (i // 4 + 1) if is_causal else NCH
            recompute = nchunks >= 2
            p16s = [[], []]
            lpartss = []

            def score_chunk(h, ncj, tag_sfx):
                ks = slice(ncj * 512, (ncj + 1) * 512)
                flush = i >= 12
                tg = "prj" if (flush and ncj % 2 == 1) else "sch"
                ps_c = psum.tile([128, 512], F32, tag=tg, bufs=2,
                                 name=f"sc_{i}_{h}_{ncj}{tag_sfx}")
                diag = is_causal and ncj == i // 4
                if diag:
                    nc.tensor.matmul(ps_c[:], ident_r[:], cmask[:, i % 4, :],
                                     start=True, stop=False)
                nc.tensor.matmul(ps_c[:], qf[:, h, i * 128:(i + 1) * 128],
                                 kf[:, ks], start=not diag, stop=True)
                if not is_causal:
                    mload = cpool.tile([128, 512], F32, tag="mload",
                                       bufs=3, name=f"ml_{i}_{h}_{ncj}{tag_sfx}")
                    nc.sync.dma_start(
                        mload[:], mask_d[i * 128:(i + 1) * 128, ks])
                    nc.vector.tensor_add(ps_c[:], ps_c[:], mload[:])
                return ps_c

            # pass 1, heads interleaved: scores -> per-chunk rowmax; psum
            # chunk banks free right after the reduce (recompute mode)
            pmxs, negms, chunks = [], [], {}
            for h in range(HPC):
                pmxs.append(cpool.tile([128, 4], F32, tag="pmx",
                                       name=f"pmx_{i}_{h}")
                            if nchunks > 1 else None)
            for ncj in range(nchunks):
                for h in range(HPC):
                    ps_c = score_chunk(h, ncj, "a")
                    if nchunks > 1:
                        nc.vector.reduce_max(pmxs[h][:, ncj:ncj + 1], ps_c[:],
                                             axis=AX)
                    if not recompute:
                        chunks[(h, ncj)] = ps_c
            for h in range(HPC):
                negm = cpool.tile([128, 1], F32, tag="negm",
                                  name=f"negm_{i}_{h}")
                if nchunks > 1:
                    nc.vector.reduce_max(negm[:], pmxs[h][:, 0:nchunks],
                                         axis=AX, negate=True)
                else:
                    nc.vector.reduce_max(negm[:], chunks[(h, 0)][:],
                                         axis=AX, negate=True)
                negms.append(negm)
            if prev[0] is not None:
                emit_tail(prev[0])
            # pass 2, heads interleaved: (recomputed) scores -> exp
            lpartss = [cpool.tile([128, 4], F32, tag="lparts",
                                  name=f"lp_{i}_{h}") for h in range(HPC)]
            for ncj in range(nchunks):
                for h in range(HPC):
                    ps_c = score_chunk(h, ncj, "b") if recompute \
                        else chunks[(h, ncj)]
                    p16 = cpool.tile([128, 512], F16, tag="p16", bufs=10,
                                     name=f"p16_{i}_{h}_{ncj}")
                    nc.scalar.activation(p16[:], ps_c[:],
                                         ACTF.Exp, bias=negms[h][:],
                                         scale=1.0,
                                         accum_out=lpartss[h][:, ncj:ncj + 1])
                    p16s[h].append(p16)
            prev[0] = (i, nchunks, p16s, lpartss)

        # ================= main schedule ================================
        for mq in range(4):
            emit_proj_chunk(2 * mq)
            if mq == 1:
                nc.sync.dma_start(
                    wo_sb[:], wo_d.rearrange("(t p) f -> p t f", p=128))
            if mq > 0:
                emit_attn_block(4 * (mq - 1))
                emit_attn_block(4 * (mq - 1) + 1)
            emit_proj_chunk(2 * mq + 1)
            if mq == 0:
                nc.sync.dma_start(cosk_sb[:], cosk_d)
                nc.sync.dma_start(sink_sb[:], sink_d)
                nc.sync.dma_start(cosq_sb[:], cosq_d)
                nc.sync.dma_start(sinq_sb[:], sinq_d)
            if mq > 0:
                emit_attn_block(4 * (mq - 1) + 2)
                emit_attn_block(4 * (mq - 1) + 3)
            emit_rope(mq)
            emit_vtrans(mq)
        for i in range(12, 16):
            emit_attn_block(i)
        emit_tail(prev[0])
        while oproj_pending:
            j, attn16 = oproj_pending.pop(0)
            ob = cpool.tile([128, H], F16, tag="ob", name=f"ob_{j}")
            emit_oproj_all([(j, attn16, ob, 0)])

    nc.compile()
    return nc


def _f32r(a):
    u = np.ascontiguousarray(a, np.float32).view(np.uint32)
    u = (u + np.uint32(0x800)) & np.uint32(0xFFFFF000)
    return u.view(np.float32)


def kernel(**inputs):
    x = np.asarray(inputs["x"], np.float32)
    cos = np.asarray(inputs["cos"], np.float32)
    sin = np.asarray(inputs["sin"], np.float32)
    am = np.asarray(inputs["attention_mask"]).reshape(S, S).astype(bool)
    wq = np.asarray(inputs["wq"], np.float32)
    bq = np.asarray(inputs["bq"], np.float32)
    wk = np.asarray(inputs["wk"], np.float32)
    bk = np.asarray(inputs["bk"], np.float32)
    wv = np.asarray(inputs["wv"], np.float32)
    bv = np.asarray(inputs["bv"], np.float32)
    wo = np.asarray(inputs["wo"], np.float32)
    bo = np.asarray(inputs["bo"], np.float32)
    qn = np.asarray(inputs["q_norm_w"], np.float32)
    kn = np.asarray(inputs["k_norm_w"], np.float32)

    assert x.shape == (1, S, H)
    is_causal = bool(
        (am == np.triu(np.ones((S, S), dtype=bool), k=1)).all())

    key = is_causal
    if key not in _prog_cache:
        _prog_cache[key] = _build(is_causal)
    nc = _prog_cache[key]

    xr = _f32r(x[0].T)
    cosT = cos.T
    sinT = sin.T
    rolled_q = np.roll(qn, -64)     # rot(q*qn)[i] = rot(q)[i] * qn[(i+64)%128]
    rolled_k = np.roll(kn, -64)
    # rope via rot'=[x2;x1] (plain swap): fold the sign of the first half
    # of sin into the table
    sgn = np.ones((HD, 1), np.float32)
    sgn[0:HD // 2] = -1.0
    cosq = np.ascontiguousarray(cosT * qn[:, None])
    sinq = np.ascontiguousarray(sinT * rolled_q[:, None] * sgn)
    cosk = np.ascontiguousarray(cosT * kn[:, None])
    sink = np.ascontiguousarray(sinT * rolled_k[:, None] * sgn)
    if not is_causal:
        maskadd = np.where(am, np.float32(NEG), np.float32(0.0))

    in_maps = []
    for c in range(NCORES):
        fq = slice(c * FQ, (c + 1) * FQ)
        g = c // 2
        fk = slice(g * HD, (g + 1) * HD)
        m = dict(
            xr=xr,
            wq=_f32r(wq[:, fq]),
            wk=_f32r(wk[:, fk]),
            wv=_f32r(wv[:, fk]),
            wo=np.ascontiguousarray(wo[fq, :].astype(np.float16)),
            cosq=cosq, sinq=sinq, cosk=cosk, sink=sink,
            bqt=np.ascontiguousarray(bq[fq].reshape(HPC, HD).T),
            bkt=np.ascontiguousarray(bk[fk].reshape(1, HD).T),
            bvt=np.ascontiguousarray(bv[fk].reshape(1, HD).T),
        )
        if not is_causal:
            m["maskadd"] = maskadd
        in_maps.append(m)

    res = bass_utils.run_bass_kernel_spmd(nc, in_maps,
                                          core_ids=list(range(NCORES)))
    acc = np.zeros((S, H), np.float64)
    for c in range(NCORES):
        acc += res.results[c]["out"]
    out = (acc + bo[None, :]).astype(np.float32)
    return out.reshape(1, S, H)
